# revision 2
# baseline (speedup 1.0000x reference)
"""GAT layer kernel for Trainium2, sharded across 8 NeuronCores.

Math: reference computes
    h = x @ W.T;  e_ij = (h @ a1)[i] + (h @ a2)[j];  mask by adj;
    softmax over j; out = attn @ h.
Because e_i is constant along the softmax axis it cancels, so with
w_j = exp(h_j . a2):
    out[i] = sum_j adj[i,j] * w_j * h[j] / sum_j adj[i,j] * w_j
a1 is mathematically irrelevant.

Design (v11, swapped phase-2 orientation):
  * adjacency is packed host-side to fp8e4 BIT PATTERNS (0x00 / 0x38 =
    1.0): 1 byte/entry (8.4 MB/core vs 33.5 int32), streamed pure-copy
    on SWDGE, interleaved just-in-time with the bf16 x stream.
  * phase 1 (fused, as v10): one loop over 32 j-super-chunks computes
    h quarters (bf16 matmuls into a 2-bank PSUM scratch), exps the e
    column straight out of PSUM, and writes the fp8 [w*h | w] tile
    (w broadcast via a stride-0 AP). ws = exp(e + ln(1/8)); the 1/8
    keeps w*h inside fp8e4 range and cancels in num/den.
  * phase 2 is TRANSPOSED vs v10: stationary = adj^T chunk
    [j:256, r:128] (fp8 DoubleRow k=256), moving = [w*h | w] [j:256,
    257 cols]. out[r, 0:256] = numerator rows, out[r, 256] = the
    denominator - it rides along as moving column 257, so the
    separate denominator matmul passes of v10 (a full third of the
    phase-2 moving cycles) vanish.
  * PSUM budget: 2 scratch banks + 6 rotating accumulator banks.
    Output rows are processed in two halves of 4 row-chunks (128 rows
    each): half A (rows 0:512) is fused with phase 1; half B (rows
    512:1024) re-reads the SBUF-resident adj tiles as stationaries in
    a pure phase-2 sweep. Half B's first two accumulators land on
    fresh banks so they don't wait on half A's epilogue reads.
  * epilogue per row-chunk: numerator and denominator share the
    partition (= output row), so the divide is cheap on-device: DVE
    reciprocal of the [128, 1] denominator column + a per-partition
    scale multiply (scalar ACT / DVE alternating) -> fp16 [128, 256],
    DMA'd straight into the final [1024, 256] layout. No host math
    beyond a concat/cast.

Measured numerics: rel err ~9.5e-3 vs fp32 reference (tolerance 2e-2).
"""

import sys

import numpy as np

for _p in ("/opt/trn_rl_repo",):
    try:
        import concourse.bass  # noqa: F401

        break
    except ImportError:
        if _p not in sys.path:
            sys.path.insert(0, _p)

import ml_dtypes

import concourse.bass as bass
import concourse.mybir as mybir
import concourse.tile as tile
from concourse.bass_utils import run_bass_kernel_spmd

dt = mybir.dt
AF = mybir.ActivationFunctionType
PM = mybir.MatmulPerfMode

N = 8192
D = 256
NCORES = 8
RB = N // NCORES  # 1024 output rows per core
W_FREE = 260  # 256 h cols + 1 e col + 3 pad
NJ = N // 128  # 64 j-chunks
NJS = N // 256  # 32 j-super-chunks (DoubleRow k=256)
NJP = NJS // 2  # 16 adj DMA transfers (2 super-chunks each)
HW_FREE = 260  # 256 w*h cols + 1 w col + 3 pad
LOG_S = float(np.log(1.0 / 8.0))  # global w scale, cancels in num/den

# ---------------------------------------------------------------------------
# walrus in this container accepts at most ONE sync-wait command on several
# instruction structs (Drain, 4-byte self-loading Matmult, ...) while the
# newer Tile scheduler emits more. Split the extras into single-wait
# EventSemaphore prefixes on the same engine (identical semantics).
_ev_counter = [0]


def _legalize_multiwait(nc, max_keep=1):
    for f in nc.m.functions:
        for bb in f.blocks:
            il = bb.instructions
            idx = 0
            while idx < len(il):
                inst = il[idx]
                si = inst.sync_info
                if si is not None and si.on_wait and len(si.on_wait) > max_keep:
                    waits = list(si.on_wait)
                    keep = waits[len(waits) - max_keep :] if max_keep else []
                    extra = waits[: len(waits) - max_keep] if max_keep else waits
                    si.on_wait = keep
                    for w in extra:
                        _ev_counter[0] += 1
                        ev = mybir.InstEventSemaphore(
                            name=f"lgw_{_ev_counter[0]}", ins=[], outs=[]
                        )
                        ev.engine = inst.engine
                        ev.sync_info = mybir.SyncInfo(on_wait=[w], on_update=[])
                        il.insert(idx, ev)
                        idx += 1
                idx += 1


# ---------------------------------------------------------------------------


def _build_program():
    nc = bass.Bass("TRN2", debug=False)

    xT = nc.dram_tensor("xT", [D, N], dt.bfloat16, kind="ExternalInput").ap()
    WTe = nc.dram_tensor("WTe", [D, W_FREE], dt.bfloat16, kind="ExternalInput").ap()
    # adj rows of this core, transposed and packed to fp8 patterns: [j, r]
    adjT8 = nc.dram_tensor("adjT8", [N, RB], dt.float8e4, kind="ExternalInput").ap()
    # final output rows of this core (fp16 is plenty: |out| <= max|h|,
    # 5e-4 rel step vs 1e-2 budget)
    outO = nc.dram_tensor("outO", [RB, D], dt.float16, kind="ExternalOutput").ap()

    XCH = 2048  # x streamed in [128, XCH] bf16 chunks
    NXB = N // XCH  # 4 chunks per i-half
    NCPB = XCH // 128  # 16 j-chunks per x chunk

    with tile.TileContext(nc) as tc:
        with (
            tc.tile_pool(name="xr", bufs=1) as xr_pool,
            tc.tile_pool(name="wte", bufs=1) as wte_pool,
            tc.tile_pool(name="hw8", bufs=1) as hw8_pool,
            tc.tile_pool(name="wcol", bufs=4) as w_pool,
            tc.tile_pool(name="adjr", bufs=16) as adj_pool,
            tc.tile_pool(name="eps", bufs=8) as ep_pool,
        ):
            wte = []
            for ic in range(2):
                t = wte_pool.tile([128, W_FREE], dt.bfloat16, name=f"wte{ic}")
                nc.scalar.dma_start(t, WTe[ic * 128 : (ic + 1) * 128, :])
                wte.append(t)

            # All bulk loads share ONE SWDGE queue so arrival order is
            # exact: first the x chunks feeding the earliest quarters
            # (b=0 split in 128KB pieces for a fast start), then the adj
            # pairs interleaved with the remaining x chunks earliest-
            # deadline-first.
            xr = [[None] * NXB for _ in range(2)]
            at_tiles = [None] * NJP

            def load_x(b):
                for ic in range(2):
                    t = xr_pool.tile(
                        [128, XCH], dt.bfloat16, name=f"xr{ic}_{b}", tag="x", bufs=4
                    )
                    rows = xT[ic * 128 : (ic + 1) * 128]
                    if b == 0:
                        for qq in range(4):
                            nc.gpsimd.dma_start(
                                t[:, qq * 512 : (qq + 1) * 512],
                                rows[:, qq * 512 : (qq + 1) * 512],
                            )
                    else:
                        nc.gpsimd.dma_start(
                            t, rows[:, b * XCH : (b + 1) * XCH]
                        )
                    xr[ic][b] = t

            def load_at(jp):
                at = adj_pool.tile(
                    [128, 2, 2, RB], dt.float8e4, name=f"at{jp}", tag="at"
                )
                src = adjT8[jp * 512 : (jp + 1) * 512, :].rearrange(
                    "(b i p) r -> p b i r", p=128, b=2
                )
                nc.gpsimd.dma_start(at, src)
                at_tiles[jp] = at

            # earliest-deadline-first: the b=0 sub-chunk 0 feeds quarters
            # 0-3, at0 feeds js 0-1, the rest follow.
            def load_x0_sub(qq):
                for ic in range(2):
                    nc.gpsimd.dma_start(
                        xr[ic][0][:, qq * 512 : (qq + 1) * 512],
                        xT[ic * 128 : (ic + 1) * 128, qq * 512 : (qq + 1) * 512],
                    )

            for ic in range(2):
                xr[ic][0] = xr_pool.tile(
                    [128, XCH], dt.bfloat16, name=f"xr{ic}_0", tag="x", bufs=4
                )
            load_x0_sub(0)
            load_at(0)
            load_x0_sub(1)
            load_at(1)
            load_x0_sub(2)
            load_x0_sub(3)
            load_at(2)
            load_at(3)
            load_x(1)
            load_at(4)
            load_at(5)
            load_x(2)
            load_at(6)
            load_at(7)
            load_x(3)
            for jp in range(8, NJP):
                load_at(jp)

            # fp8 moving tile for phase 2:
            # hw8_all[:, jc, d] = (w*h/8)[j = jc*128 + p, d]  for d < 256
            # hw8_all[:, jc, 256] = (w/8)[j]   (the denominator column)
            hw8_all = hw8_pool.tile([128, NJ, HW_FREE], dt.float8e4, name="hw8_all")
            w_all = w_pool.tile([128, NJ], dt.float32, name="w_all")
            bias_s = w_pool.tile([128, 1], dt.float32, name="bias_s")
            nc.vector.memset(bias_s, LOG_S)

            # ---- fused loop: PSUM = 2 scratch banks (ph_q) + 6 rotating
            # accumulator banks = exactly 8.
            with (
                tc.tile_pool(name="ph", bufs=1, space="PSUM") as ph_pool,
                tc.tile_pool(name="acc", bufs=1, space="PSUM") as acc_pool,
            ):
                ph_q = ph_pool.tile([128, 2, 512], dt.float32, name="ph_q")
                # HAM warm-up: dependency-free matmuls on uninitialized SBUF
                # while the first x/adj DMAs are in flight, so the PE clock
                # gate is already at 8/8 when real work starts. Garbage
                # results land in ph_q slot 0 and are overwritten by the
                # first real start=True matmul.
                warm = hw8_pool.tile([128, 640], dt.bfloat16, name="warm")
                # memset on DVE: the gpsimd queue is busy emitting all the
                # SWDGE DMA descriptors for ~30us - anything queued behind
                # them would delay the warm-up matmuls (and the whole PE
                # timeline) by that much.
                nc.vector.memset(warm, 0.0)
                for _ in range(10):
                    nc.tensor.matmul(
                        ph_q[:, 0, :],
                        warm[:, 0:128],
                        warm[:, 128:640],
                        start=True,
                        stop=True,
                    )

                def emit_quarter(q):
                    # h/e matmuls for j-chunks 2q, 2q+1 into the scratch
                    # banks, then ws = exp(e)/8 and the fp8 moving tile.
                    for ic in range(2):
                        for k in range(2):
                            jc = 2 * q + k
                            b, sl = jc // NCPB, bass.ts(jc % NCPB, 128)
                            nc.tensor.matmul(
                                ph_q[:, k, 0:W_FREE],
                                xr[ic][b][:, sl],
                                wte[ic],
                                start=(ic == 0),
                                stop=(ic == 1),
                            )
                    j0 = 2 * q
                    nc.scalar.activation(
                        w_all[:, j0 : j0 + 2],
                        ph_q[:, :, 256],
                        AF.Exp,
                        bias=bias_s[:, 0:1],
                    )
                    nc.scalar.activation(
                        hw8_all[:, j0 : j0 + 2, 256], w_all[:, j0 : j0 + 2], AF.Copy
                    )
                    nc.vector.tensor_tensor(
                        hw8_all[:, j0 : j0 + 2, 0:256],
                        ph_q[:, :, 0:256],
                        w_all[:, j0 : j0 + 2].to_broadcast([128, 2, 256]),
                        mybir.AluOpType.mult,
                    )

                def emit_js(js, accs, rc0):
                    # phase-2 matmuls for super-chunk js, row-chunks
                    # rc0..rc0+3: stationary = adj^T [256, 128], moving =
                    # [w*h | w] [256, 257]; out[r, 0:256]=num, out[r,256]=den
                    at = at_tiles[js // 2][:, js % 2]  # [128, 2, RB]
                    st, sp = js == 0, js == NJS - 1
                    mov = hw8_all[:, 2 * js : 2 * js + 2, 0:257]
                    for k, acc in enumerate(accs):
                        rc = rc0 + k
                        nc.tensor.matmul(
                            acc[:, 0:257],
                            at[:, :, rc * 128 : (rc + 1) * 128],
                            mov,
                            start=st,
                            stop=sp,
                            perf_mode=PM.DoubleRow,
                            skip_group_check=True,
                        )

                def emit_epilogue(accs, rc0):
                    # num and den share the partition (= output row): DVE
                    # reciprocal of the [128,1] den column, then a
                    # per-partition scale multiply -> fp16, straight out.
                    for k, acc in enumerate(accs):
                        rc = rc0 + k
                        rec = ep_pool.tile([128, 1], dt.float32, name="rec", tag="rec")
                        nc.vector.reciprocal(rec, acc[:, 256:257])
                        ob = ep_pool.tile([128, 256], dt.float16, name="ob", tag="ob")
                        if k % 2 == 0:
                            nc.scalar.activation(
                                ob, acc[:, 0:256], AF.Copy, scale=rec[:, 0:1]
                            )
                        else:
                            nc.vector.tensor_scalar_mul(ob, acc[:, 0:256], rec[:, 0:1])
                        eng = nc.sync if k % 2 == 0 else nc.scalar
                        eng.dma_start(outO[rc * 128 : (rc + 1) * 128, :], ob)

                # ---- half A: rows 0:512 fused with phase 1
                accA = [
                    acc_pool.tile([128, 512], dt.float32, name=f"accA{rc}",
                                  tag="acc", bufs=6)
                    for rc in range(4)
                ]
                for q in range(NJS):
                    emit_quarter(q)
                    if q >= 1:
                        emit_js(q - 1, accA, 0)
                emit_js(NJS - 1, accA, 0)
                emit_epilogue(accA, 0)

                # ---- half B: rows 512:1024, pure phase-2 sweep; the first
                # two accumulators rotate onto fresh banks, the last two
                # reuse accA[0]/accA[1] whose epilogue reads finish first.
                accB = [
                    acc_pool.tile([128, 512], dt.float32, name=f"accB{rc}",
                                  tag="acc", bufs=6)
                    for rc in range(4)
                ]
                for js in range(NJS):
                    emit_js(js, accB, 4)
                emit_epilogue(accB, 4)

    _legalize_multiwait(nc, max_keep=1)
    return nc


_CACHED = {}


def _prep_inputs(x, adj, W, a):
    xT = np.ascontiguousarray(x.T).astype(ml_dtypes.bfloat16)
    WTe = np.zeros((D, W_FREE), dtype=np.float32)
    WTe[:, :256] = W.T
    WTe[:, 256] = (W.T.astype(np.float64) @ a[256:].astype(np.float64)).astype(
        np.float32
    )
    WTe = WTe.astype(ml_dtypes.bfloat16)
    # adjacency -> fp8e4 bit patterns (0x00 / 0x38 == 1.0), transposed per core
    adj8 = np.where(adj != 0, np.uint8(0x38), np.uint8(0)).view(ml_dtypes.float8_e4m3)
    in_maps = []
    for c in range(NCORES):
        adjT8_c = np.ascontiguousarray(adj8[c * RB : (c + 1) * RB, :].T)
        in_maps.append({"xT": xT, "WTe": WTe, "adjT8": adjT8_c})
    return in_maps


def _run(in_maps, **kw):
    if "nc" not in _CACHED:
        _CACHED["nc"] = _build_program()
    # The device occasionally comes up wedged (NRT_EXEC_UNIT_UNRECOVERABLE)
    # from a previous process; one retry after a short pause recovers it.
    import time as _time

    last_err = None
    for attempt in range(3):
        try:
            return run_bass_kernel_spmd(
                _CACHED["nc"], in_maps, core_ids=list(range(NCORES)), **kw
            )
        except Exception as e:  # noqa: BLE001
            last_err = e
            if "UNRECOVERABLE" not in str(e) and "UNAVAILABLE" not in str(e):
                raise
            _time.sleep(3.0)
    raise last_err


def _assemble(results):
    blocks = [np.asarray(r["outO"], dtype=np.float32) for r in results]
    return np.concatenate(blocks, axis=0)


def kernel(x, adj, W, a):
    in_maps = _prep_inputs(x, adj, W, a)
    res = _run(in_maps)
    return _assemble(res.results)


# revision 3
# speedup vs baseline: 1.3121x; 1.3121x over previous
"""GAT layer kernel for Trainium2, sharded across 8 NeuronCores.

Math: reference computes
    h = x @ W.T;  e_ij = (h @ a1)[i] + (h @ a2)[j];  mask by adj;
    softmax over j; out = attn @ h.
Because e_i is constant along the softmax axis it cancels, so with
w_j = exp(h_j . a2):
    out[i] = sum_j adj[i,j] * w_j * h[j] / sum_j adj[i,j] * w_j
a1 is mathematically irrelevant.

Design (v12, swapped phase-2 orientation + pipelined scratch):
  * adjacency is packed host-side to fp8e4 BIT PATTERNS (0x00 / 0x38 =
    1.0), PRE-ARRANGED per 512-row super-tile into the exact [p, b, i,
    r] device layout so every adj DMA is a pure contiguous copy (4KB
    descriptors - descriptor emission on the SWDGE ring would
    otherwise cap arrival below phase-2's consumption rate).
  * phase 1: h quarters (2 j-chunks) via bf16 matmuls into a PSUM
    scratch that ROTATES over two 2-bank buffers, so the exp/convert
    chain of quarter q drains while quarter q+1 computes. Phase-2
    consumption lags two super-chunks behind. The chain is split
    across engines: batched Exp on scalar ACT, the (w*h)->fp8 scale
    multiply alternates scalar ACT (per-partition scale operand) and
    DVE tensor_scalar, the fp8 w column is a DVE cast copy.
    ws = exp(e + ln(1/8)); the 1/8 keeps w*h inside fp8e4 range and
    cancels in num/den.
  * phase 2 is TRANSPOSED vs v10: stationary = adj^T chunk [j:256,
    r:128] (fp8 DoubleRow k=256), moving = [w*h | w] [j:256, 257
    cols]. out[r, 0:256] = numerator rows, out[r, 256] = the
    denominator - it rides along as moving column 257, so v10's
    separate denominator matmuls (a third of the phase-2 moving
    cycles) vanish.
  * PSUM budget: 4 scratch banks + 4 rotating accumulator banks.
    Output rows go in two halves of 4 row-chunks: half A (rows 0:512)
    fused with phase 1; half B (rows 512:1024) a pure phase-2 sweep
    re-reading the SBUF-resident adj tiles. Half B reuses half A's
    accumulator banks right after the (fast, early) epilogue-A reads.
  * epilogue per row-chunk: numerator and denominator share the
    partition (= output row), so the divide is cheap on-device: DVE
    reciprocal of the [128, 1] denominator column + a per-partition
    scale multiply -> fp16 [128, 256], DMA'd straight into the final
    [1024, 256] layout. No host math beyond a concat/cast.

Measured numerics: rel err ~9.7e-3 vs fp32 reference (tolerance 2e-2).
"""

import sys

import numpy as np

for _p in ("/opt/trn_rl_repo",):
    try:
        import concourse.bass  # noqa: F401

        break
    except ImportError:
        if _p not in sys.path:
            sys.path.insert(0, _p)

import ml_dtypes

import concourse.bass as bass
import concourse.mybir as mybir
import concourse.tile as tile
from concourse.bass_utils import run_bass_kernel_spmd

dt = mybir.dt
AF = mybir.ActivationFunctionType
PM = mybir.MatmulPerfMode

N = 8192
D = 256
NCORES = 8
RB = N // NCORES  # 1024 output rows per core
W_FREE = 260  # 256 h cols + 1 e col + 3 pad
NJ = N // 128  # 64 j-chunks
NJS = N // 256  # 32 j-super-chunks (DoubleRow k=256)
NJP = NJS // 2  # 16 adj DMA transfers (2 super-chunks each)
HW_FREE = 260  # 256 w*h cols + 1 w col + 3 pad
LOG_S = float(np.log(1.0 / 8.0))  # global w scale, cancels in num/den

# ---------------------------------------------------------------------------
# walrus in this container accepts at most ONE sync-wait command on several
# instruction structs (Drain, 4-byte self-loading Matmult, ...) while the
# newer Tile scheduler emits more. Split the extras into single-wait
# EventSemaphore prefixes on the same engine (identical semantics).
_ev_counter = [0]


def _legalize_multiwait(nc, max_keep=1):
    for f in nc.m.functions:
        for bb in f.blocks:
            il = bb.instructions
            idx = 0
            while idx < len(il):
                inst = il[idx]
                si = inst.sync_info
                if si is not None and si.on_wait and len(si.on_wait) > max_keep:
                    waits = list(si.on_wait)
                    keep = waits[len(waits) - max_keep :] if max_keep else []
                    extra = waits[: len(waits) - max_keep] if max_keep else waits
                    si.on_wait = keep
                    for w in extra:
                        _ev_counter[0] += 1
                        ev = mybir.InstEventSemaphore(
                            name=f"lgw_{_ev_counter[0]}", ins=[], outs=[]
                        )
                        ev.engine = inst.engine
                        ev.sync_info = mybir.SyncInfo(on_wait=[w], on_update=[])
                        il.insert(idx, ev)
                        idx += 1
                idx += 1


# ---------------------------------------------------------------------------


def _build_program():
    nc = bass.Bass("TRN2", debug=False)

    xT = nc.dram_tensor("xT", [D, N], dt.bfloat16, kind="ExternalInput").ap()
    WTe = nc.dram_tensor("WTe", [D, W_FREE], dt.bfloat16, kind="ExternalInput").ap()
    # adj rows of this core, packed host-side into the device tile layout:
    # adjP8[jp, p, b, i, r] = adj^T fp8 pattern for j = jp*512 + b*256 +
    # i*128 + p; each [p, b, i, r] tile is one contiguous 512KB DMA.
    adjP8 = nc.dram_tensor(
        "adjP8", [NJP, 128, 2, 2, RB], dt.float8e4, kind="ExternalInput"
    ).ap()
    # final output rows of this core (fp16 is plenty: |out| <= max|h|,
    # 5e-4 rel step vs 1e-2 budget)
    outO = nc.dram_tensor("outO", [RB, D], dt.float16, kind="ExternalOutput").ap()

    XCH = 2048  # x streamed in [128, XCH] bf16 chunks
    NXB = N // XCH  # 4 chunks per i-half
    NCPB = XCH // 128  # 16 j-chunks per x chunk

    with tile.TileContext(nc) as tc:
        with (
            tc.tile_pool(name="xr", bufs=1) as xr_pool,
            tc.tile_pool(name="wte", bufs=1) as wte_pool,
            tc.tile_pool(name="hw8", bufs=1) as hw8_pool,
            tc.tile_pool(name="wcol", bufs=4) as w_pool,
            tc.tile_pool(name="adjr", bufs=16) as adj_pool,
            tc.tile_pool(name="eps", bufs=8) as ep_pool,
        ):
            wte = []
            for ic in range(2):
                t = wte_pool.tile([128, W_FREE], dt.bfloat16, name=f"wte{ic}")
                nc.scalar.dma_start(t, WTe[ic * 128 : (ic + 1) * 128, :])
                wte.append(t)

            # All bulk loads share ONE SWDGE queue so arrival order is
            # exact: first the x chunks feeding the earliest quarters
            # (b=0 split in 128KB pieces for a fast start), then the adj
            # tiles interleaved with the remaining x chunks earliest-
            # deadline-first.
            xr = [[None] * NXB for _ in range(2)]
            at_tiles = [None] * NJP

            def load_x(b):
                for ic in range(2):
                    t = xr_pool.tile(
                        [128, XCH], dt.bfloat16, name=f"xr{ic}_{b}", tag="x", bufs=4
                    )
                    rows = xT[ic * 128 : (ic + 1) * 128]
                    if b == 0:
                        for qq in range(4):
                            nc.gpsimd.dma_start(
                                t[:, qq * 512 : (qq + 1) * 512],
                                rows[:, qq * 512 : (qq + 1) * 512],
                            )
                    else:
                        nc.gpsimd.dma_start(
                            t, rows[:, b * XCH : (b + 1) * XCH]
                        )
                    xr[ic][b] = t

            def load_at(jp, split=1):
                at = adj_pool.tile(
                    [128, 2, 2, RB], dt.float8e4, name=f"at{jp}", tag="at"
                )
                src = adjP8[jp].rearrange("p b i r -> p (b i r)")
                dst = at.rearrange("p b i r -> p (b i r)")
                step = (2 * 2 * RB) // split
                for s in range(split):
                    nc.gpsimd.dma_start(
                        dst[:, s * step : (s + 1) * step],
                        src[:, s * step : (s + 1) * step],
                    )
                at_tiles[jp] = at

            # earliest-deadline-first: the b=0 sub-chunk 0 feeds quarters
            # 0-3, at0 feeds js 0-1, the rest follow.
            def load_x0_sub(qq):
                for ic in range(2):
                    nc.gpsimd.dma_start(
                        xr[ic][0][:, qq * 512 : (qq + 1) * 512],
                        xT[ic * 128 : (ic + 1) * 128, qq * 512 : (qq + 1) * 512],
                    )

            for ic in range(2):
                xr[ic][0] = xr_pool.tile(
                    [128, XCH], dt.bfloat16, name=f"xr{ic}_0", tag="x", bufs=4
                )
            load_x0_sub(0)
            load_at(0, split=2)
            load_x0_sub(1)
            load_at(1, split=2)
            load_x0_sub(2)
            load_x0_sub(3)
            load_at(2)
            load_at(3)
            load_x(1)
            load_at(4)
            load_at(5)
            load_x(2)
            load_at(6)
            load_at(7)
            load_x(3)
            for jp in range(8, NJP):
                load_at(jp)

            # fp8 moving tile for phase 2:
            # hw8_all[:, jc, d] = (w*h/8)[j = jc*128 + p, d]  for d < 256
            # hw8_all[:, jc, 256] = (w/8)[j]   (the denominator column)
            hw8_all = hw8_pool.tile([128, NJ, HW_FREE], dt.float8e4, name="hw8_all")
            w_all = w_pool.tile([128, NJ], dt.float32, name="w_all")
            bias_s = w_pool.tile([128, 1], dt.float32, name="bias_s")
            nc.vector.memset(bias_s, LOG_S)

            # ---- fused loop: PSUM = 2x2 rotating scratch banks + 4
            # rotating accumulator banks = exactly 8.
            with (
                tc.tile_pool(name="ph", bufs=1, space="PSUM") as ph_pool,
                tc.tile_pool(name="acc", bufs=1, space="PSUM") as acc_pool,
            ):
                ph_bufs = [
                    ph_pool.tile([128, 2, 512], dt.float32, name=f"ph_q{i}",
                                 tag="ph", bufs=2)
                    for i in range(2)
                ]
                # HAM warm-up: dependency-free matmuls on uninitialized SBUF
                # while the first x/adj DMAs are in flight, so the PE clock
                # gate is already at 8/8 when real work starts. Garbage
                # results land in ph buf 0 and are overwritten by the
                # first real start=True matmul.
                warm = hw8_pool.tile([128, 640], dt.bfloat16, name="warm")
                # memset on DVE: the gpsimd queue is busy emitting the SWDGE
                # DMA descriptors - anything queued behind them would delay
                # the warm-up matmuls (and the whole PE timeline).
                nc.vector.memset(warm, 0.0)
                for _ in range(10):
                    nc.tensor.matmul(
                        ph_bufs[0][:, 0, :],
                        warm[:, 0:128],
                        warm[:, 128:640],
                        start=True,
                        stop=True,
                    )

                def emit_quarter(q):
                    # h/e matmuls for j-chunks 2q, 2q+1 into the rotating
                    # scratch, then ws = exp(e)/8 and the fp8 moving tile.
                    ph_q = ph_bufs[q % 2]
                    for ic in range(2):
                        for k in range(2):
                            jc = 2 * q + k
                            b, sl = jc // NCPB, bass.ts(jc % NCPB, 128)
                            nc.tensor.matmul(
                                ph_q[:, k, 0:W_FREE],
                                xr[ic][b][:, sl],
                                wte[ic],
                                start=(ic == 0),
                                stop=(ic == 1),
                            )
                    j0 = 2 * q
                    nc.scalar.activation(
                        w_all[:, j0 : j0 + 2],
                        ph_q[:, :, 256],
                        AF.Exp,
                        bias=bias_s[:, 0:1],
                    )
                    # fp8 w column (denominator) - cheap cast copy on DVE
                    nc.vector.tensor_copy(
                        hw8_all[:, j0 : j0 + 2, 256], w_all[:, j0 : j0 + 2]
                    )
                    # (w*h) -> fp8, one j-chunk per engine so neither the
                    # scalar nor the vector engine becomes the bottleneck
                    nc.scalar.activation(
                        hw8_all[:, j0, 0:256],
                        ph_q[:, 0, 0:256],
                        AF.Copy,
                        scale=w_all[:, j0 : j0 + 1],
                    )
                    nc.vector.tensor_scalar(
                        hw8_all[:, j0 + 1, 0:256],
                        ph_q[:, 1, 0:256],
                        w_all[:, j0 + 1 : j0 + 2],
                        None,
                        mybir.AluOpType.mult,
                    )

                def emit_js(js, accs, rc0):
                    # phase-2 matmuls for super-chunk js, row-chunks
                    # rc0..rc0+3: stationary = adj^T [256, 128], moving =
                    # [w*h | w] [256, 257]; out[r, 0:256]=num, out[r,256]=den
                    at = at_tiles[js // 2][:, js % 2]  # [128, 2, RB]
                    st, sp = js == 0, js == NJS - 1
                    mov = hw8_all[:, 2 * js : 2 * js + 2, 0:257]
                    for k, acc in enumerate(accs):
                        rc = rc0 + k
                        nc.tensor.matmul(
                            acc[:, 0:257],
                            at[:, :, rc * 128 : (rc + 1) * 128],
                            mov,
                            start=st,
                            stop=sp,
                            perf_mode=PM.DoubleRow,
                            skip_group_check=True,
                        )

                def emit_epilogue(accs, rc0):
                    # num and den share the partition (= output row): DVE
                    # reciprocal of the [128,1] den column, then a
                    # per-partition scale multiply -> fp16, straight out.
                    for k, acc in enumerate(accs):
                        rc = rc0 + k
                        rec = ep_pool.tile([128, 1], dt.float32, name="rec", tag="rec")
                        nc.vector.reciprocal(rec, acc[:, 256:257])
                        ob = ep_pool.tile([128, 256], dt.float16, name="ob", tag="ob")
                        if k % 2 == 0:
                            nc.scalar.activation(
                                ob, acc[:, 0:256], AF.Copy, scale=rec[:, 0:1]
                            )
                        else:
                            nc.vector.tensor_scalar_mul(ob, acc[:, 0:256], rec[:, 0:1])
                        eng = nc.sync if k % 2 == 0 else nc.scalar
                        eng.dma_start(outO[rc * 128 : (rc + 1) * 128, :], ob)

                # ---- half A: rows 0:512 fused with phase 1; phase-2
                # consumption lags two super-chunks so the convert chain
                # never stalls the PE.
                accA = [
                    acc_pool.tile([128, 512], dt.float32, name=f"accA{rc}",
                                  tag="acc", bufs=4)
                    for rc in range(4)
                ]
                for q in range(NJS):
                    if q >= 2:
                        emit_js(q - 2, accA, 0)
                    emit_quarter(q)
                emit_js(NJS - 2, accA, 0)
                emit_js(NJS - 1, accA, 0)
                emit_epilogue(accA, 0)

                # ---- half B: rows 512:1024, pure phase-2 sweep; the
                # accumulators rotate onto half A's banks, whose epilogue
                # reads are already done (they were emitted first).
                accB = [
                    acc_pool.tile([128, 512], dt.float32, name=f"accB{rc}",
                                  tag="acc", bufs=4)
                    for rc in range(4)
                ]
                for js in range(NJS):
                    emit_js(js, accB, 4)
                emit_epilogue(accB, 4)

    _legalize_multiwait(nc, max_keep=1)
    return nc


_CACHED = {}


def _prep_inputs(x, adj, W, a):
    xT = np.ascontiguousarray(x.T).astype(ml_dtypes.bfloat16)
    WTe = np.zeros((D, W_FREE), dtype=np.float32)
    WTe[:, :256] = W.T
    WTe[:, 256] = (W.T.astype(np.float64) @ a[256:].astype(np.float64)).astype(
        np.float32
    )
    WTe = WTe.astype(ml_dtypes.bfloat16)
    # adjacency -> fp8e4 bit patterns (0x00 / 0x38 == 1.0), pre-arranged
    # per core into the [jp, p, b, i, r] device tile layout (j = jp*512 +
    # b*256 + i*128 + p) so every adj DMA is a contiguous copy.
    adj8 = np.where(adj != 0, np.uint8(0x38), np.uint8(0))
    in_maps = []
    for c in range(NCORES):
        adjT_c = np.ascontiguousarray(adj8[c * RB : (c + 1) * RB, :].T)  # [N, RB]
        adjP = adjT_c.reshape(NJP, 2, 2, 128, RB).transpose(0, 3, 1, 2, 4)
        adjP = np.ascontiguousarray(adjP).view(ml_dtypes.float8_e4m3)
        in_maps.append({"xT": xT, "WTe": WTe, "adjP8": adjP})
    return in_maps


def _run(in_maps, **kw):
    if "nc" not in _CACHED:
        _CACHED["nc"] = _build_program()
    # The device occasionally comes up wedged (NRT_EXEC_UNIT_UNRECOVERABLE)
    # from a previous process; one retry after a short pause recovers it.
    import time as _time

    last_err = None
    for attempt in range(3):
        try:
            return run_bass_kernel_spmd(
                _CACHED["nc"], in_maps, core_ids=list(range(NCORES)), **kw
            )
        except Exception as e:  # noqa: BLE001
            last_err = e
            if "UNRECOVERABLE" not in str(e) and "UNAVAILABLE" not in str(e):
                raise
            _time.sleep(3.0)
    raise last_err


def _assemble(results):
    blocks = [np.asarray(r["outO"], dtype=np.float32) for r in results]
    return np.concatenate(blocks, axis=0)


def kernel(x, adj, W, a):
    in_maps = _prep_inputs(x, adj, W, a)
    res = _run(in_maps)
    return _assemble(res.results)
